# revision 8
# baseline (speedup 1.0000x reference)
"""DTW layer kernel for Trainium2 (8 NeuronCores, Bass/Tile).

Problem: weighted DTW with w = RHO**(1/L_PATTS) = 1.0 (RHO=1.0), so
    D[i,j] = cost[i,j] + min(D[i-1,j-1], D[i-1,j], D[i,j-1])
with cost[b,n,i,j] = ||patts[n,:,i] - x[b,:,j]||^2, output D[..., -32:].

Sharding: data-parallel over batch B (4 batches/core x 32 patterns
= 128 independent DP chains per core = the 128 SBUF partitions).

Per core:
  - cost row i computed by one fused matmul: stationary [96,128] =
    [-2*patts_blockdiag ; ones_blockdiag], moving [96,1024] = [x ; x^2],
    contracting K=(b,d) twice -> (-2*cross + x2) in PSUM; p2[n,i] added
    during PSUM->SBUF eviction.
  - DP runs row-major: row i is a min-plus scan along j:
      D_i[j] = min(m[j], D_i[j-1]) + c[j],  m[j] = min(D_{i-1}[j-1], D_{i-1}[j])
    -> one DVE tensor_tensor(min) + one DVE tensor_tensor_scan per row.
"""

import sys

sys.path.insert(0, "/opt/trn_rl_repo")

import numpy as np

import concourse.bass as bass
import concourse.tile as tile
from concourse import bacc, mybir
from concourse.bass_utils import run_bass_kernel_spmd

B, N, D, P, L = 32, 32, 12, 32, 1024
LOUT = 32
BIG = 1e30
NCORES = 8
BPC = B // NCORES  # batches per core
KD = BPC * D  # 48: stacked (b, d) contraction rows
F32 = mybir.dt.float32

_cached_nc = {}


def _build_kernel(reps: int = 1):
    if reps in _cached_nc:
        return _cached_nc[reps]

    nc = bacc.Bacc("TRN2", target_bir_lowering=False, debug=False, num_devices=NCORES)
    x_d = nc.dram_tensor("x", [KD, L], F32, kind="ExternalInput").ap()
    patts_d = nc.dram_tensor("patts", [N, D, P], F32, kind="ExternalInput").ap()
    out_d = nc.dram_tensor("out", [128, P * LOUT], F32, kind="ExternalOutput").ap()

    with tile.TileContext(nc) as tc:
        with (
            tc.tile_pool(name="singles", bufs=1) as singles,
            tc.tile_pool(name="psum", bufs=2, space="PSUM") as psum,
            tc.tile_pool(name="cost", bufs=3) as costp,
            tc.tile_pool(name="mp", bufs=2) as mp,
        ):
            # ---- persistent tiles
            # K layout (128 rows): 0-47 = x (b,d), 48-63 zero, 64-111 = x^2,
            # 112-127 zero. Engine ops need 32-aligned partition starts.
            xstack = singles.tile([128, L], F32)  # [x ; pad ; x^2 ; pad]
            stat = singles.tile([128, 128, P], F32)  # matmul stationary
            ones_stage = singles.tile([D, N, P], F32)  # 1.0 staging block
            prep = singles.tile([128, D, P], F32)  # patts replicated (n,d,i)
            p2 = singles.tile([128, P], F32)  # sum_d patts^2 per (n,i)
            m0 = singles.tile([128, L], F32)  # row-0 "m": [0, BIG, ...]
            dA = singles.tile([128, L + 1], F32)  # D row ping
            dB = singles.tile([128, L + 1], F32)  # D row pong
            outs = singles.tile([128, P, LOUT], F32)

            # ---- prologue
            nc.vector.memset(xstack, 0.0)
            nc.sync.dma_start(out=xstack[0:KD, :], in_=x_d[:, :])
            nc.vector.tensor_mul(xstack[64 : 64 + KD, :], xstack[0:KD, :], xstack[0:KD, :])

            nc.vector.memset(stat, 0.0)
            nc.vector.memset(ones_stage, 1.0)
            for b in range(BPC):
                # patts[n,d,i] -> stat[12b+d, 32b+n, i] (transposed d<->n view)
                src = bass.AP(
                    tensor=patts_d.tensor,
                    offset=patts_d.offset,
                    ap=[[P, D], [D * P, N], [1, P]],
                )
                nc.sync.dma_start(
                    out=stat[D * b : D * (b + 1), N * b : N * (b + 1), :], in_=src
                )
            nc.vector.tensor_scalar_mul(stat[0:KD], stat[0:KD], -2.0)
            for b in range(BPC):
                # ones block for the x^2 contraction rows (DMA: engine memset
                # can't start at unaligned partitions)
                nc.sync.dma_start(
                    out=stat[64 + D * b : 64 + D * (b + 1), N * b : N * (b + 1), :],
                    in_=ones_stage[:, :, :],
                )

            for b in range(BPC):
                # patts[n,d,i] -> prep[32b+n, d, i] (contiguous copy)
                nc.sync.dma_start(out=prep[N * b : N * (b + 1), :, :], in_=patts_d[:, :, :])
            nc.vector.tensor_mul(prep, prep, prep)
            # reduce over d via a free-dim-permuted view [128, i, d]
            prep_view = prep[:, :, :].rearrange("p d i -> p i d")
            nc.vector.tensor_reduce(p2, prep_view, axis=mybir.AxisListType.X, op=mybir.AluOpType.add)

            nc.vector.memset(m0, BIG)
            nc.vector.memset(m0[:, 0:1], 0.0)
            nc.vector.memset(dA[:, 0:1], BIG)
            nc.vector.memset(dB[:, 0:1], BIG)

            # ---- DP over rows (reps>1 only for wall-clock slope timing)
            for _rep in range(reps):
                dprev, dcur = dA, dB
                for i in range(P):
                    ptile = psum.tile([128, L], F32)
                    for h in range(2):
                        nc.tensor.matmul(
                            out=ptile[:, 512 * h : 512 * (h + 1)],
                            lhsT=stat[:, :, i],
                            rhs=xstack[:, 512 * h : 512 * (h + 1)],
                            start=True,
                            stop=True,
                        )
                    ctile = costp.tile([128, L], F32)
                    nc.scalar.activation(ctile, ptile, func=mybir.ActivationFunctionType.Copy)
                    nc.gpsimd.tensor_scalar_add(ctile, ctile, p2[:, i : i + 1])

                    if i == 0:
                        m_ap = m0[:, :]
                    else:
                        mtile = mp.tile([128, L], F32)
                        nc.vector.tensor_tensor(
                            mtile, dprev[:, 0:L], dprev[:, 1 : L + 1], op=mybir.AluOpType.min
                        )
                        m_ap = mtile[:, :]
                    nc.vector.tensor_tensor_scan(
                        out=dcur[:, 1 : L + 1],
                        data0=m_ap,
                        data1=ctile[:, :],
                        initial=BIG,
                        op0=mybir.AluOpType.min,
                        op1=mybir.AluOpType.add,
                    )
                    nc.gpsimd.tensor_copy(outs[:, i, :], dcur[:, L + 1 - LOUT : L + 1])
                    dprev, dcur = dcur, dprev

            nc.sync.dma_start(out=out_d[:, :], in_=outs[:, :, :])

    nc.compile()
    _cached_nc[reps] = nc
    return nc


def kernel(x: np.ndarray, patts: np.ndarray) -> np.ndarray:
    nc = _build_kernel()
    patts_np = np.ascontiguousarray(patts, dtype=np.float32)
    in_maps = []
    for c in range(NCORES):
        xc = np.ascontiguousarray(
            x[BPC * c : BPC * (c + 1)], dtype=np.float32
        ).reshape(KD, L)
        in_maps.append({"x": xc, "patts": patts_np})
    res = run_bass_kernel_spmd(nc, in_maps, list(range(NCORES)))
    parts = [res.results[c]["out"].reshape(BPC, N, P, LOUT) for c in range(NCORES)]
    return np.concatenate(parts, axis=0)


# revision 9
# speedup vs baseline: 2.2757x; 2.2757x over previous
"""DTW layer kernel for Trainium2 (8 NeuronCores, Bass/Tile).

Problem: weighted DTW with w = RHO**(1/L_PATTS) = 1.0 (RHO=1.0), so
    D[i,j] = cost[i,j] + min(D[i-1,j-1], D[i-1,j], D[i,j-1])
with cost[b,n,i,j] = ||patts[n,:,i] - x[b,:,j]||^2, output D[..., -32:].

Sharding: data-parallel over batch B (4 batches/core x 32 patterns
= 128 independent DP chains per core = the 128 SBUF partitions).

The execution target is per-instruction-overhead dominated, so the
design minimizes instruction count:
  - cost row i = 2 matmuls straight into PSUM. Stationary K-rows fold
    the whole cost expression: rows 0-47 = -2*patts (block-diag over
    the 4 local batches), rows 64-111 = block-diag ones against x^2
    (-> +x2 term), row 112 = p2[n,i] against a constant-1 row of the
    moving operand (-> +p2 term).
  - DP row-major: one DVE tensor_tensor(min) for
    m[j]=min(D[i-1,j-1],D[i-1,j]) + one DVE tensor_tensor_scan
    (op0=min, op1=add) reading cost directly from PSUM:
      D_i[j] = min(m[j], D_i[j-1]) + c[j]
  - all D rows kept in one SBUF tensor; single strided output DMA.
"""

import sys

sys.path.insert(0, "/opt/trn_rl_repo")

import numpy as np

import concourse.bass as bass
import concourse.tile as tile
from concourse import bacc, mybir
from concourse.bass_utils import run_bass_kernel_spmd

B, N, D, P, L = 32, 32, 12, 32, 1024
LOUT = 32
BIG = 1e30
NCORES = 8
BPC = B // NCORES  # batches per core
KD = BPC * D  # 48 stacked (b, d) contraction rows
F32 = mybir.dt.float32

_cached_nc = {}


def _build_kernel(reps: int = 1):
    if reps in _cached_nc:
        return _cached_nc[reps]

    nc = bacc.Bacc("TRN2", target_bir_lowering=False, debug=False, num_devices=NCORES)
    x_d = nc.dram_tensor("x", [KD, L], F32, kind="ExternalInput").ap()
    patts_d = nc.dram_tensor("patts", [N, D, P], F32, kind="ExternalInput").ap()
    out_d = nc.dram_tensor("out", [128, P * LOUT], F32, kind="ExternalOutput").ap()

    with tile.TileContext(nc) as tc:
        with (
            tc.tile_pool(name="singles", bufs=1) as singles,
            tc.tile_pool(name="psum", bufs=2, space="PSUM") as psum,
        ):
            # K layout (128 rows): 0-47 x, 48-63 const 1.0 (unused: stat=0),
            # 64-111 x^2, 112-127 const 1.0 (row 112 pairs with p2 in stat).
            xstack = singles.tile([128, L], F32)
            stat = singles.tile([128, 128, P], F32)  # [K, m=(b,n), i]
            ones_stage = singles.tile([D, N, P], F32)
            pst = singles.tile([D, N, P], F32)  # patts in [d, n, i]
            ones12 = singles.tile([D, 1], F32)
            p2row = singles.tile([1, N * P], F32)
            m0 = singles.tile([128, L], F32)  # row-0 m: [0, BIG, ...]
            mbuf = singles.tile([128, L], F32)
            dfull = singles.tile([128, P, L + 1], F32)  # all DP rows, col0 = BIG pad

            # ---- prologue
            nc.vector.memset(xstack, 1.0)
            nc.sync.dma_start(out=xstack[0:KD, :], in_=x_d[:, :])
            nc.vector.tensor_mul(xstack[64 : 64 + KD, :], xstack[0:KD, :], xstack[0:KD, :])

            nc.vector.memset(stat, 0.0)
            nc.vector.memset(ones_stage, 1.0)
            nc.vector.memset(ones12, 1.0)
            # patts[n,d,i] -> [d, n, i] staging (also reused per-batch-block)
            patts_T = bass.AP(
                tensor=patts_d.tensor,
                offset=patts_d.offset,
                ap=[[P, D], [D * P, N], [1, P]],
            )
            nc.sync.dma_start(out=pst, in_=patts_T)
            for b in range(BPC):
                nc.sync.dma_start(
                    out=stat[D * b : D * (b + 1), N * b : N * (b + 1), :], in_=patts_T
                )
            nc.vector.tensor_scalar_mul(stat[0:KD], stat[0:KD], -2.0)
            for b in range(BPC):
                # block-diag ones rows for the x^2 contraction (DMA: engine
                # memset can't start at unaligned partitions)
                nc.sync.dma_start(
                    out=stat[64 + D * b : 64 + D * (b + 1), N * b : N * (b + 1), :],
                    in_=ones_stage[:, :, :],
                )
            # p2[n,i] = sum_d patts^2 -> stat row 112, replicated per batch block
            nc.vector.tensor_mul(pst, pst, pst)
            p2p = psum.tile([1, N * P], F32)
            for h in range(2):
                nc.tensor.matmul(
                    out=p2p[:, 512 * h : 512 * (h + 1)],
                    lhsT=ones12,
                    rhs=pst[:, :, :].rearrange("p n i -> p (n i)")[:, 512 * h : 512 * (h + 1)],
                    start=True,
                    stop=True,
                )
            nc.vector.tensor_copy(p2row, p2p)
            for b in range(BPC):
                nc.sync.dma_start(
                    out=stat[112:113, N * b : N * (b + 1), :],
                    in_=p2row[:, :].rearrange("p (n i) -> p n i", n=N),
                )

            nc.vector.memset(m0, BIG)
            nc.vector.memset(m0[:, 0:1], 0.0)
            nc.vector.memset(dfull[:, :, 0:1], BIG)

            # ---- DP over rows (reps>1 only for wall-clock slope timing)
            for _rep in range(reps):
                for i in range(P):
                    ptile = psum.tile([128, L], F32)
                    for h in range(2):
                        nc.tensor.matmul(
                            out=ptile[:, 512 * h : 512 * (h + 1)],
                            lhsT=stat[:, :, i],
                            rhs=xstack[:, 512 * h : 512 * (h + 1)],
                            start=True,
                            stop=True,
                        )
                    if i == 0:
                        m_ap = m0[:, :]
                    else:
                        nc.vector.tensor_tensor(
                            mbuf,
                            dfull[:, i - 1, 0:L],
                            dfull[:, i - 1, 1 : L + 1],
                            op=mybir.AluOpType.min,
                        )
                        m_ap = mbuf[:, :]
                    nc.vector.tensor_tensor_scan(
                        out=dfull[:, i, 1 : L + 1],
                        data0=m_ap,
                        data1=ptile[:, :],
                        initial=BIG,
                        op0=mybir.AluOpType.min,
                        op1=mybir.AluOpType.add,
                    )

            nc.sync.dma_start(out=out_d[:, :], in_=dfull[:, :, L + 1 - LOUT : L + 1])

    nc.compile()
    _cached_nc[reps] = nc
    return nc


def kernel(x: np.ndarray, patts: np.ndarray) -> np.ndarray:
    nc = _build_kernel()
    patts_np = np.ascontiguousarray(patts, dtype=np.float32)
    in_maps = []
    for c in range(NCORES):
        xc = np.ascontiguousarray(
            x[BPC * c : BPC * (c + 1)], dtype=np.float32
        ).reshape(KD, L)
        in_maps.append({"x": xc, "patts": patts_np})
    res = run_bass_kernel_spmd(nc, in_maps, list(range(NCORES)))
    parts = [res.results[c]["out"].reshape(BPC, N, P, LOUT) for c in range(NCORES)]
    return np.concatenate(parts, axis=0)


# revision 12
# speedup vs baseline: 2.8195x; 1.2390x over previous
"""DTW layer kernel for Trainium2 (8 NeuronCores, Bass/Tile).

Problem: weighted DTW with w = RHO**(1/L_PATTS) = 1.0 (RHO=1.0), so
    D[i,j] = cost[i,j] + min(D[i-1,j-1], D[i-1,j], D[i,j-1])
with cost[b,n,i,j] = ||patts[n,:,i] - x[b,:,j]||^2, output D[..., -32:].

Sharding: data-parallel over batch B (4 batches/core x 32 patterns
= 128 independent DP chains per core = the 128 SBUF partitions).

The execution target is per-instruction-overhead dominated, so the
design minimizes instruction count:
  - cost row i = 2 matmuls straight into PSUM. Stationary K-rows fold
    the whole cost expression: rows 0-47 = -2*patts (block-diag over
    the 4 local batches), rows 64-111 = block-diag ones against x^2
    (-> +x2 term), row 112 = p2[n,i] against a constant-1 row of the
    moving operand (-> +p2 term).
  - DP row-major: one DVE tensor_tensor(min) for
    m[j]=min(D[i-1,j-1],D[i-1,j]) + one DVE tensor_tensor_scan
    (op0=min, op1=add) reading cost directly from PSUM:
      D_i[j] = min(m[j], D_i[j-1]) + c[j]
  - all D rows kept in one SBUF tensor; single strided output DMA.
"""

import sys

sys.path.insert(0, "/opt/trn_rl_repo")

import numpy as np

import concourse.bass as bass
import concourse.tile as tile
from concourse import bacc, mybir
from concourse.bass_utils import run_bass_kernel_spmd

# ---------------------------------------------------------------------------
# Hand-assembled custom DVE op: fused DTW row update at 1 elem/cycle.
#
# Per partition, for j = 0..N-1 (streams: Src0 = Dprev, Src1 = c):
#     dd[j] = Dprev[j-1]            (swap-flop delay; dd[0] = s0 = BIG)
#     m[j]  = min(dd[j], Dprev[j])
#     S[j]  = S[j-1] + c[j]         (scan-add, init 0)
#     t[j]  = m[j] - S[j-1]
#     r[j]  = min(r[j-1], t[j])     (scan-min, init s0 = BIG)
#     out[j] = S[j] + r[j]
# which equals the DTW row recurrence
#     out[j] = c[j] + min(Dprev[j-1], Dprev[j], out[j-1]),  out[-1] = BIG.
# ---------------------------------------------------------------------------
from concourse.dve_spec import Spec, Src0, Src1, C0, C1, scan, AluOp as SAluOp
from concourse.dve_spec import _has_src1 as has_src1
from concourse.dve_uop import (
    UopConfig,
    UopDpConfig,
    AluOp,
    AluInp,
    DelayInp,
    InpSel,
    OutSel,
    OutPath,
    Trigger,
    DveOpSpec,
    ENABLE,
    DISABLE,
)
import concourse.dve_ops as dve_ops


def _dtw_row_reference(in0, in1, s0, s1, imm2):
    """numpy semantics for CoreSim: in0=Dprev [P,N], in1=c [P,N], s0=BIG."""
    Pp, Nn = in0.shape
    big = np.broadcast_to(np.asarray(s0, np.float32), (Pp,)).astype(np.float32)
    dprev_sh = np.concatenate([big[:, None], in0[:, :-1]], axis=1)
    m = np.minimum(dprev_sh, in0)
    out = np.empty_like(in0)
    state = big.copy()
    for j in range(Nn):
        state = np.minimum(m[:, j], state) + in1[:, j]
        out[:, j] = state
    return out


def _dtw_steady() -> UopConfig:
    u = UopConfig()
    u.enable_input(InpSel.SRC_0, 1)  # chain0 = Dprev[j]
    u.enable_input(InpSel.SRC_1, 2)  # chain1 = c[j]
    u.enable_input(InpSel.CONST_0, 3)  # chain2 = BIG (seed reads)
    u.enable_input(InpSel.ZERO, 4)  # chain3 = 0.0 (seed reads)
    u.require_inp0 = ENABLE
    u.require_inp1 = ENABLE
    u.repeat_count = 0
    u.trigger = (Trigger.SRC_TENSOR_DONE, Trigger.NONE, Trigger.NONE)
    u.next_uop = (0, 0, 0)
    u.enable_output(OutSel.ALU_OUT, OutPath.WR0_LO)

    dp = u.datapath_config
    # b0: delay register via swap flop (BYPASS outputs a=old swap, captures b)
    dp[0].enable_alu(AluOp.BYPASS, AluInp.CURR_SWAP_OUT, AluInp.PREV_DELAY_0)
    dp[0].swap_enable = ENABLE
    dp[0].pass_through_delay(0, 1, 2, 3)
    # b1: m = min(dd, Dprev[j])
    dp[1].enable_alu(AluOp.MIN, AluInp.PREV_ALU_OUT, AluInp.PREV_DELAY_0)
    dp[1].pass_through_delay(1, 2, 3)
    # b2: S = S + c (feedback); capture m into chain4
    dp[2].enable_alu(AluOp.ADD, AluInp.CURR_ALU_OUT, AluInp.PREV_DELAY_1)
    dp[2].pass_through_delay(1, 2)
    dp[2].enable_delay_from_src(DelayInp.PREV_ALU_OUT, 4)
    # b3: Sx = S - c (= S[j-1]); capture S into chain5
    dp[3].enable_alu(AluOp.SUBTRACT, AluInp.PREV_ALU_OUT, AluInp.PREV_DELAY_1)
    dp[3].pass_through_delay(2, 4)
    dp[3].enable_delay_from_src(DelayInp.PREV_ALU_OUT, 5)
    # b4: t = m - S[j-1]
    dp[4].enable_alu(AluOp.SUBTRACT, AluInp.PREV_DELAY_4, AluInp.PREV_ALU_OUT)
    dp[4].pass_through_delay(2, 5)
    # b5: r = min(r, t) (feedback)
    dp[5].enable_alu(AluOp.MIN, AluInp.CURR_ALU_OUT, AluInp.PREV_ALU_OUT)
    dp[5].pass_through_delay(2, 5)
    # b6: out = r + S
    dp[6].enable_alu(AluOp.ADD, AluInp.PREV_ALU_OUT, AluInp.PREV_DELAY_5)
    # b7: passthrough to the output mux
    dp[7].pass_through_alu()
    return u


def _dtw_seed() -> UopConfig:
    u = UopConfig()
    u.enable_input(InpSel.SRC_0, 1)
    u.enable_input(InpSel.SRC_1, 2)
    u.enable_input(InpSel.CONST_0, 3)
    u.enable_input(InpSel.ZERO, 4)
    u.require_inp0 = DISABLE
    u.require_inp1 = DISABLE
    u.repeat_count = 1
    u.trigger = (Trigger.COUNT, Trigger.NONE, Trigger.NONE)
    u.next_uop = (1, 0, 0)

    dp = u.datapath_config
    # b0: swap <- C0 (BIG): BYPASS captures operand b into the swap flop
    dp[0].enable_alu(AluOp.BYPASS, AluInp.PREV_DELAY_2, AluInp.PREV_DELAY_2)
    dp[0].swap_enable = ENABLE
    dp[0].pass_through_delay(0, 1, 2, 3)
    dp[1].pass_through_alu()
    dp[1].pass_through_delay(1, 2, 3)
    # b2: S-state <- 0.0 (chain3)
    dp[2].enable_alu(AluOp.BYPASS, AluInp.PREV_DELAY_3, AluInp.PREV_DELAY_3)
    dp[2].pass_through_delay(1, 2)
    dp[3].pass_through_alu()
    dp[3].pass_through_delay(2)
    dp[4].pass_through_alu()
    dp[4].pass_through_delay(2)
    # b5: r-state <- C0 (BIG)
    dp[5].enable_alu(AluOp.BYPASS, AluInp.PREV_DELAY_2, AluInp.PREV_DELAY_2)
    dp[6].pass_through_alu()
    dp[7].pass_through_alu()
    return u


class _HandDveOp:
    """DveOp stand-in: compile() returns the hand-assembled DveOpSpec."""

    def __init__(self, name, spec, uops):
        self.name = name
        self.spec = spec
        self.subdim = False
        self.perf_en = {}
        self._uops = uops
        self._cache = {}

    def compile(self, ver):
        if ver not in self._cache:
            self._cache[ver] = DveOpSpec(
                name=self.name,
                opcode=dve_ops.get_dve_sub_opcode(self.name),
                uops=self._uops,
                rd1_en=has_src1(self.spec),
            )
        return self._cache[ver]


_REGISTERED = {}


def _register_dtw_row_op():
    if "DTW_ROW_ANT" in _REGISTERED:
        return _REGISTERED["DTW_ROW_ANT"]
    # representative spec: correct leaves {Src0, Src1, C0} + numpy reference
    S = scan(SAluOp.ADD, Src1)
    rep_body = S + scan(SAluOp.MIN, Src0 - C1, init=C0)
    spec = Spec(body=rep_body, reference=_dtw_row_reference)
    name = "DTW_ROW_ANT"
    op = _HandDveOp(name, spec, [_dtw_seed(), _dtw_steady()])
    dve_ops.OPS.append(op)
    dve_ops.CUSTOM_DVE_SPECS[name] = spec
    dve_ops._SUB_OPCODE_FOR_NAME[name] = (
        dve_ops._CUSTOM_DVE_ROW_BASE + len(dve_ops.OPS) - 1
    )
    _REGISTERED[name] = op
    return op


B, N, D, P, L = 32, 32, 12, 32, 1024
LOUT = 32
BIG = 1e30
NCORES = 8
BPC = B // NCORES  # batches per core
KD = BPC * D  # 48 stacked (b, d) contraction rows
F32 = mybir.dt.float32

_cached_nc = {}


def _build_kernel(reps: int = 1):
    if reps in _cached_nc:
        return _cached_nc[reps]

    nc = bacc.Bacc("TRN2", target_bir_lowering=False, debug=False, num_devices=NCORES)
    x_d = nc.dram_tensor("x", [KD, L], F32, kind="ExternalInput").ap()
    patts_d = nc.dram_tensor("patts", [N, D, P], F32, kind="ExternalInput").ap()
    out_d = nc.dram_tensor("out", [128, P * LOUT], F32, kind="ExternalOutput").ap()

    with tile.TileContext(nc) as tc:
        with (
            tc.tile_pool(name="singles", bufs=1) as singles,
            tc.tile_pool(name="psum", bufs=2, space="PSUM") as psum,
        ):
            # K layout (128 rows): 0-47 x, 48-63 const 1.0 (unused: stat=0),
            # 64-111 x^2, 112-127 const 1.0 (row 112 pairs with p2 in stat).
            xstack = singles.tile([128, L], F32)
            stat = singles.tile([128, 128, P], F32)  # [K, m=(b,n), i]
            ones_stage = singles.tile([D, N, P], F32)
            pst = singles.tile([D, N, P], F32)  # patts in [d, n, i]
            ones12 = singles.tile([D, 1], F32)
            p2row = singles.tile([1, N * P], F32)
            m0 = singles.tile([128, L], F32)  # row-0 m: [0, BIG, ...]
            dfull = singles.tile([128, P, L], F32)  # all DP rows

            # ---- prologue
            nc.vector.memset(xstack, 1.0)
            nc.sync.dma_start(out=xstack[0:KD, :], in_=x_d[:, :])
            nc.vector.tensor_mul(xstack[64 : 64 + KD, :], xstack[0:KD, :], xstack[0:KD, :])

            nc.vector.memset(stat, 0.0)
            nc.vector.memset(ones_stage, 1.0)
            nc.vector.memset(ones12, 1.0)
            # patts[n,d,i] -> [d, n, i] staging (also reused per-batch-block)
            patts_T = bass.AP(
                tensor=patts_d.tensor,
                offset=patts_d.offset,
                ap=[[P, D], [D * P, N], [1, P]],
            )
            nc.sync.dma_start(out=pst, in_=patts_T)
            for b in range(BPC):
                nc.sync.dma_start(
                    out=stat[D * b : D * (b + 1), N * b : N * (b + 1), :], in_=patts_T
                )
            nc.vector.tensor_scalar_mul(stat[0:KD], stat[0:KD], -2.0)
            for b in range(BPC):
                # block-diag ones rows for the x^2 contraction (DMA: engine
                # memset can't start at unaligned partitions)
                nc.sync.dma_start(
                    out=stat[64 + D * b : 64 + D * (b + 1), N * b : N * (b + 1), :],
                    in_=ones_stage[:, :, :],
                )
            # p2[n,i] = sum_d patts^2 -> stat row 112, replicated per batch block
            nc.vector.tensor_mul(pst, pst, pst)
            p2p = psum.tile([1, N * P], F32)
            for h in range(2):
                nc.tensor.matmul(
                    out=p2p[:, 512 * h : 512 * (h + 1)],
                    lhsT=ones12,
                    rhs=pst[:, :, :].rearrange("p n i -> p (n i)")[:, 512 * h : 512 * (h + 1)],
                    start=True,
                    stop=True,
                )
            nc.vector.tensor_copy(p2row, p2p)
            for b in range(BPC):
                nc.sync.dma_start(
                    out=stat[112:113, N * b : N * (b + 1), :],
                    in_=p2row[:, :].rearrange("p (n i) -> p n i", n=N),
                )

            nc.vector.memset(m0, BIG)
            nc.vector.memset(m0[:, 0:1], 0.0)

            dtw_op = _register_dtw_row_op()

            # ---- DP over rows (reps>1 only for wall-clock slope timing)
            for _rep in range(reps):
                for i in range(P):
                    ptile = psum.tile([128, L], F32)
                    for h in range(2):
                        nc.tensor.matmul(
                            out=ptile[:, 512 * h : 512 * (h + 1)],
                            lhsT=stat[:, :, i],
                            rhs=xstack[:, 512 * h : 512 * (h + 1)],
                            start=True,
                            stop=True,
                        )
                    if i == 0:
                        # row 0: m = [0, BIG, ...] constant; plain scan
                        nc.vector.tensor_tensor_scan(
                            out=dfull[:, 0, :],
                            data0=m0[:, :],
                            data1=ptile[:, :],
                            initial=BIG,
                            op0=mybir.AluOpType.min,
                            op1=mybir.AluOpType.add,
                        )
                    else:
                        # fused row update: min-shift + min-plus scan in one op
                        nc.vector._custom_dve(
                            dtw_op,
                            out=dfull[:, i, :],
                            in0=dfull[:, i - 1, :],
                            in1=ptile[:, :],
                            s0=BIG,
                            s1=0.0,
                        )

            nc.sync.dma_start(out=out_d[:, :], in_=dfull[:, :, L - LOUT : L])

    nc.compile()
    _cached_nc[reps] = nc
    return nc


def kernel(x: np.ndarray, patts: np.ndarray) -> np.ndarray:
    nc = _build_kernel()
    patts_np = np.ascontiguousarray(patts, dtype=np.float32)
    in_maps = []
    for c in range(NCORES):
        xc = np.ascontiguousarray(
            x[BPC * c : BPC * (c + 1)], dtype=np.float32
        ).reshape(KD, L)
        in_maps.append({"x": xc, "patts": patts_np})
    res = run_bass_kernel_spmd(nc, in_maps, list(range(NCORES)))
    parts = [res.results[c]["out"].reshape(BPC, N, P, LOUT) for c in range(NCORES)]
    return np.concatenate(parts, axis=0)


# revision 15
# speedup vs baseline: 3.5410x; 1.2559x over previous
"""DTW layer kernel for Trainium2 (8 NeuronCores, Bass/Tile).

Problem: weighted DTW with w = RHO**(1/L_PATTS) = 1.0 (RHO=1.0), so
    D[i,j] = cost[i,j] + min(D[i-1,j-1], D[i-1,j], D[i,j-1])
with cost[b,n,i,j] = ||patts[n,:,i] - x[b,:,j]||^2, output D[..., -32:].

Sharding: data-parallel over batch B (4 batches/core x 32 patterns
= 128 independent DP chains per core = the 128 SBUF partitions).

The execution target is per-instruction-overhead dominated, so the
design minimizes instruction count:
  - cost row i = 2 matmuls straight into PSUM. Stationary K-rows fold
    the whole cost expression: rows 0-47 = -2*patts (block-diag over
    the 4 local batches), rows 64-111 = block-diag ones against x^2
    (-> +x2 term), row 112 = p2[n,i] against a constant-1 row of the
    moving operand (-> +p2 term).
  - DP row-major: one DVE tensor_tensor(min) for
    m[j]=min(D[i-1,j-1],D[i-1,j]) + one DVE tensor_tensor_scan
    (op0=min, op1=add) reading cost directly from PSUM:
      D_i[j] = min(m[j], D_i[j-1]) + c[j]
  - all D rows kept in one SBUF tensor; single strided output DMA.
"""

import sys

sys.path.insert(0, "/opt/trn_rl_repo")

import numpy as np

import concourse.bass as bass
import concourse.tile as tile
from concourse import bacc, mybir
from concourse.bass_utils import run_bass_kernel_spmd

# ---------------------------------------------------------------------------
# Hand-assembled custom DVE op: fused DTW row update at 1 elem/cycle.
#
# Per partition, for j = 0..N-1 (streams: Src0 = Dprev, Src1 = c):
#     dd[j] = Dprev[j-1]            (swap-flop delay; dd[0] = s0 = BIG)
#     m[j]  = min(dd[j], Dprev[j])
#     S[j]  = S[j-1] + c[j]         (scan-add, init 0)
#     t[j]  = m[j] - S[j-1]
#     r[j]  = min(r[j-1], t[j])     (scan-min, init s0 = BIG)
#     out[j] = S[j] + r[j]
# which equals the DTW row recurrence
#     out[j] = c[j] + min(Dprev[j-1], Dprev[j], out[j-1]),  out[-1] = BIG.
# ---------------------------------------------------------------------------
from concourse.dve_spec import Spec, Src0, Src1, C0, C1, scan, AluOp as SAluOp
from concourse.dve_spec import _has_src1 as has_src1
from concourse.dve_uop import (
    UopConfig,
    UopDpConfig,
    AluOp,
    AluInp,
    DelayInp,
    InpSel,
    OutSel,
    OutPath,
    Trigger,
    DveOpSpec,
    ENABLE,
    DISABLE,
)
import concourse.dve_ops as dve_ops


def _dtw_row_reference(in0, in1, s0, s1, imm2):
    """numpy semantics for CoreSim: in0=Dprev [P,N], in1=c [P,N], s0=BIG."""
    Pp, Nn = in0.shape
    big = np.broadcast_to(np.asarray(s0, np.float32), (Pp,)).astype(np.float32)
    dprev_sh = np.concatenate([big[:, None], in0[:, :-1]], axis=1)
    m = np.minimum(dprev_sh, in0)
    out = np.empty_like(in0)
    state = big.copy()
    for j in range(Nn):
        state = np.minimum(m[:, j], state) + in1[:, j]
        out[:, j] = state
    return out


def _dtw_steady() -> UopConfig:
    u = UopConfig()
    u.enable_input(InpSel.SRC_0, 1)  # chain0 = Dprev[j]
    u.enable_input(InpSel.SRC_1, 2)  # chain1 = c[j]
    u.enable_input(InpSel.CONST_0, 3)  # chain2 = BIG (seed reads)
    u.enable_input(InpSel.ZERO, 4)  # chain3 = 0.0 (seed reads)
    u.require_inp0 = ENABLE
    u.require_inp1 = ENABLE
    u.repeat_count = 0
    u.trigger = (Trigger.SRC_TENSOR_DONE, Trigger.NONE, Trigger.NONE)
    u.next_uop = (0, 0, 0)
    u.enable_output(OutSel.ALU_OUT, OutPath.WR0_LO)

    dp = u.datapath_config
    # b0: delay register via swap flop (BYPASS outputs a=old swap, captures b)
    dp[0].enable_alu(AluOp.BYPASS, AluInp.CURR_SWAP_OUT, AluInp.PREV_DELAY_0)
    dp[0].swap_enable = ENABLE
    dp[0].pass_through_delay(0, 1, 2, 3)
    # b1: m = min(dd, Dprev[j])
    dp[1].enable_alu(AluOp.MIN, AluInp.PREV_ALU_OUT, AluInp.PREV_DELAY_0)
    dp[1].pass_through_delay(1, 2, 3)
    # b2: S = S + c (feedback); capture m into chain4
    dp[2].enable_alu(AluOp.ADD, AluInp.CURR_ALU_OUT, AluInp.PREV_DELAY_1)
    dp[2].pass_through_delay(1, 2)
    dp[2].enable_delay_from_src(DelayInp.PREV_ALU_OUT, 4)
    # b3: Sx = S - c (= S[j-1]); capture S into chain5
    dp[3].enable_alu(AluOp.SUBTRACT, AluInp.PREV_ALU_OUT, AluInp.PREV_DELAY_1)
    dp[3].pass_through_delay(2, 4)
    dp[3].enable_delay_from_src(DelayInp.PREV_ALU_OUT, 5)
    # b4: t = m - S[j-1]
    dp[4].enable_alu(AluOp.SUBTRACT, AluInp.PREV_DELAY_4, AluInp.PREV_ALU_OUT)
    dp[4].pass_through_delay(2, 5)
    # b5: r = min(r, t) (feedback)
    dp[5].enable_alu(AluOp.MIN, AluInp.CURR_ALU_OUT, AluInp.PREV_ALU_OUT)
    dp[5].pass_through_delay(2, 5)
    # b6: out = r + S
    dp[6].enable_alu(AluOp.ADD, AluInp.PREV_ALU_OUT, AluInp.PREV_DELAY_5)
    # b7: passthrough to the output mux
    dp[7].pass_through_alu()
    return u


def _dtw_seed() -> UopConfig:
    u = UopConfig()
    u.enable_input(InpSel.SRC_0, 1)
    u.enable_input(InpSel.SRC_1, 2)
    u.enable_input(InpSel.CONST_0, 3)
    u.enable_input(InpSel.ZERO, 4)
    u.require_inp0 = DISABLE
    u.require_inp1 = DISABLE
    u.repeat_count = 1
    u.trigger = (Trigger.COUNT, Trigger.NONE, Trigger.NONE)
    u.next_uop = (1, 0, 0)

    dp = u.datapath_config
    # b0: swap <- C0 (BIG): BYPASS captures operand b into the swap flop
    dp[0].enable_alu(AluOp.BYPASS, AluInp.PREV_DELAY_2, AluInp.PREV_DELAY_2)
    dp[0].swap_enable = ENABLE
    dp[0].pass_through_delay(0, 1, 2, 3)
    dp[1].pass_through_alu()
    dp[1].pass_through_delay(1, 2, 3)
    # b2: S-state <- 0.0 (chain3)
    dp[2].enable_alu(AluOp.BYPASS, AluInp.PREV_DELAY_3, AluInp.PREV_DELAY_3)
    dp[2].pass_through_delay(1, 2)
    dp[3].pass_through_alu()
    dp[3].pass_through_delay(2)
    dp[4].pass_through_alu()
    dp[4].pass_through_delay(2)
    # b5: r-state <- C0 (BIG)
    dp[5].enable_alu(AluOp.BYPASS, AluInp.PREV_DELAY_2, AluInp.PREV_DELAY_2)
    dp[6].pass_through_alu()
    dp[7].pass_through_alu()
    return u


class _HandDveOp:
    """DveOp stand-in: compile() returns the hand-assembled DveOpSpec."""

    def __init__(self, name, spec, uops):
        self.name = name
        self.spec = spec
        self.subdim = False
        self.perf_en = {}
        self._uops = uops
        self._cache = {}

    def compile(self, ver):
        if ver not in self._cache:
            self._cache[ver] = DveOpSpec(
                name=self.name,
                opcode=dve_ops.get_dve_sub_opcode(self.name),
                uops=self._uops,
                rd1_en=has_src1(self.spec),
            )
        return self._cache[ver]


_REGISTERED = {}


def _register_dtw_row_op():
    if "DTW_ROW_ANT" in _REGISTERED:
        return _REGISTERED["DTW_ROW_ANT"]
    # representative spec: correct leaves {Src0, Src1, C0} + numpy reference
    S = scan(SAluOp.ADD, Src1)
    rep_body = S + scan(SAluOp.MIN, Src0 - C1, init=C0)
    spec = Spec(body=rep_body, reference=_dtw_row_reference)
    name = "DTW_ROW_ANT"
    op = _HandDveOp(name, spec, [_dtw_seed(), _dtw_steady()])
    dve_ops.OPS.append(op)
    dve_ops.CUSTOM_DVE_SPECS[name] = spec
    dve_ops._SUB_OPCODE_FOR_NAME[name] = (
        dve_ops._CUSTOM_DVE_ROW_BASE + len(dve_ops.OPS) - 1
    )
    _REGISTERED[name] = op
    return op


B, N, D, P, L = 32, 32, 12, 32, 1024
LOUT = 32
BIG = 1e30
NCORES = 8
BPC = B // NCORES  # batches per core
KD = BPC * D  # 48 stacked (b, d) contraction rows
F32 = mybir.dt.float32

_cached_nc = {}


def _build_kernel(reps: int = 1):
    if reps in _cached_nc:
        return _cached_nc[reps]

    nc = bacc.Bacc("TRN2", target_bir_lowering=False, debug=False, num_devices=NCORES)
    x_d = nc.dram_tensor("x", [KD, L], F32, kind="ExternalInput").ap()
    patts_d = nc.dram_tensor("patts", [N, D, P], F32, kind="ExternalInput").ap()
    out_d = nc.dram_tensor("out", [128, P * LOUT], F32, kind="ExternalOutput").ap()

    with tile.TileContext(nc) as tc:
        with (
            tc.tile_pool(name="singles", bufs=1) as singles,
            tc.tile_pool(name="psum4", bufs=1, space="PSUM") as psum4,
        ):
            # K layout (128 rows): 0-47 x, 48-63 const 1.0 (unused: stat=0),
            # 64-111 x^2, 112-127 const 1.0 (row 112 pairs with p2 in stat).
            xstack = singles.tile([128, L], F32)
            stat = singles.tile([128, 128, P], F32)  # [K, m=(b,n), i]
            ones_stage = singles.tile([D, N, P], F32)
            pst = singles.tile([D, N, P], F32)  # patts in [d, n, i]
            ones12 = singles.tile([D, 1], F32)
            p2row = singles.tile([1, N * P], F32)
            m0 = singles.tile([128, L], F32)  # row-0 m: [0, BIG, ...]
            dfull = singles.tile([128, P, L], F32)  # all DP rows

            # ---- prologue
            nc.vector.memset(xstack, 1.0)
            nc.sync.dma_start(out=xstack[0:KD, :], in_=x_d[:, :])
            nc.vector.tensor_mul(xstack[64 : 64 + KD, :], xstack[0:KD, :], xstack[0:KD, :])

            nc.vector.memset(stat, 0.0)
            nc.vector.memset(ones_stage, 1.0)
            nc.vector.memset(ones12, 1.0)
            # patts[n,d,i] -> [d, n, i] staging (also reused per-batch-block)
            patts_T = bass.AP(
                tensor=patts_d.tensor,
                offset=patts_d.offset,
                ap=[[P, D], [D * P, N], [1, P]],
            )
            nc.sync.dma_start(out=pst, in_=patts_T)
            for b in range(BPC):
                nc.sync.dma_start(
                    out=stat[D * b : D * (b + 1), N * b : N * (b + 1), :], in_=patts_T
                )
            nc.vector.tensor_scalar_mul(stat[0:KD], stat[0:KD], -2.0)
            for b in range(BPC):
                # block-diag ones rows for the x^2 contraction (DMA: engine
                # memset can't start at unaligned partitions)
                nc.sync.dma_start(
                    out=stat[64 + D * b : 64 + D * (b + 1), N * b : N * (b + 1), :],
                    in_=ones_stage[:, :, :],
                )
            # p2[n,i] = sum_d patts^2 -> stat row 112, replicated per batch block
            nc.vector.tensor_mul(pst, pst, pst)
            p2p = psum4.tile([1, N * P], F32, tag="pt")
            for h in range(2):
                nc.tensor.matmul(
                    out=p2p[:, 512 * h : 512 * (h + 1)],
                    lhsT=ones12,
                    rhs=pst[:, :, :].rearrange("p n i -> p (n i)")[:, 512 * h : 512 * (h + 1)],
                    start=True,
                    stop=True,
                )
            nc.vector.tensor_copy(p2row, p2p)
            for b in range(BPC):
                nc.sync.dma_start(
                    out=stat[112:113, N * b : N * (b + 1), :],
                    in_=p2row[:, :].rearrange("p (n i) -> p n i", n=N),
                )

            nc.vector.memset(m0, BIG)
            nc.vector.memset(m0[:, 0:1], 0.0)

            dtw_op = _register_dtw_row_op()

            # ---- DP over rows, batched 4 rows per PSUM fill (8 banks) to
            # amortize cross-engine semaphores (reps>1 only for slope timing)
            GB = 4
            for _rep in range(reps):
                for g in range(P // GB):
                    ptile = psum4.tile([128, GB, L], F32, tag="pt")
                    for r in range(GB):
                        for h in range(2):
                            nc.tensor.matmul(
                                out=ptile[:, r, 512 * h : 512 * (h + 1)],
                                lhsT=stat[:, :, GB * g + r],
                                rhs=xstack[:, 512 * h : 512 * (h + 1)],
                                start=True,
                                stop=True,
                            )
                    for r in range(GB):
                        i = GB * g + r
                        if i == 0:
                            # row 0: m = [0, BIG, ...] constant; plain scan
                            nc.vector.tensor_tensor_scan(
                                out=dfull[:, 0, :],
                                data0=m0[:, :],
                                data1=ptile[:, 0, :],
                                initial=BIG,
                                op0=mybir.AluOpType.min,
                                op1=mybir.AluOpType.add,
                            )
                        else:
                            # fused row update: min-shift + min-plus scan, one op
                            nc.vector._custom_dve(
                                dtw_op,
                                out=dfull[:, i, :],
                                in0=dfull[:, i - 1, :],
                                in1=ptile[:, r, :],
                                s0=BIG,
                                s1=0.0,
                            )

            nc.sync.dma_start(out=out_d[:, :], in_=dfull[:, :, L - LOUT : L])

    nc.compile()
    _cached_nc[reps] = nc
    return nc


def kernel(x: np.ndarray, patts: np.ndarray) -> np.ndarray:
    nc = _build_kernel()
    patts_np = np.ascontiguousarray(patts, dtype=np.float32)
    in_maps = []
    for c in range(NCORES):
        xc = np.ascontiguousarray(
            x[BPC * c : BPC * (c + 1)], dtype=np.float32
        ).reshape(KD, L)
        in_maps.append({"x": xc, "patts": patts_np})
    res = run_bass_kernel_spmd(nc, in_maps, list(range(NCORES)))
    parts = [res.results[c]["out"].reshape(BPC, N, P, LOUT) for c in range(NCORES)]
    return np.concatenate(parts, axis=0)


# revision 17
# speedup vs baseline: 5.9222x; 1.6725x over previous
"""DTW layer kernel for Trainium2 (8 NeuronCores, Bass/Tile).

Problem: weighted DTW with w = RHO**(1/L_PATTS) = 1.0 (RHO=1.0), so
    D[i,j] = cost[i,j] + min(D[i-1,j-1], D[i-1,j], D[i,j-1])
with cost[b,n,i,j] = ||patts[n,:,i] - x[b,:,j]||^2, output D[..., -32:].

Sharding: data-parallel over batch B (4 batches/core x 32 patterns
= 128 independent DP chains per core = the 128 SBUF partitions).

The execution target is per-instruction-overhead dominated, so the
design minimizes instruction count:
  - cost row i = 2 matmuls straight into PSUM. Stationary K-rows fold
    the whole cost expression: rows 0-47 = -2*patts (block-diag over
    the 4 local batches), rows 64-111 = block-diag ones against x^2
    (-> +x2 term), row 112 = p2[n,i] against a constant-1 row of the
    moving operand (-> +p2 term).
  - DP row-major: one DVE tensor_tensor(min) for
    m[j]=min(D[i-1,j-1],D[i-1,j]) + one DVE tensor_tensor_scan
    (op0=min, op1=add) reading cost directly from PSUM:
      D_i[j] = min(m[j], D_i[j-1]) + c[j]
  - all D rows kept in one SBUF tensor; single strided output DMA.
"""

import sys

sys.path.insert(0, "/opt/trn_rl_repo")

import numpy as np

import concourse.bass as bass
import concourse.tile as tile
from concourse import bacc, mybir
from concourse.bass_utils import run_bass_kernel_spmd

# ---------------------------------------------------------------------------
# Hand-assembled custom DVE op: fused DTW row update at 1 elem/cycle.
#
# Per partition, for j = 0..N-1 (streams: Src0 = Dprev, Src1 = c):
#     dd[j] = Dprev[j-1]            (swap-flop delay; dd[0] = s0 = BIG)
#     m[j]  = min(dd[j], Dprev[j])
#     S[j]  = S[j-1] + c[j]         (scan-add, init 0)
#     t[j]  = m[j] - S[j-1]
#     r[j]  = min(r[j-1], t[j])     (scan-min, init s0 = BIG)
#     out[j] = S[j] + r[j]
# which equals the DTW row recurrence
#     out[j] = c[j] + min(Dprev[j-1], Dprev[j], out[j-1]),  out[-1] = BIG.
# ---------------------------------------------------------------------------
from concourse.dve_spec import Spec, Src0, Src1, C0, C1, scan, AluOp as SAluOp
from concourse.dve_spec import _has_src1 as has_src1
from concourse.dve_uop import (
    UopConfig,
    UopDpConfig,
    AluOp,
    AluInp,
    DelayInp,
    InpSel,
    OutSel,
    OutPath,
    Trigger,
    DveOpSpec,
    ENABLE,
    DISABLE,
)
import concourse.dve_ops as dve_ops


def _dtw_row_reference(in0, in1, s0, s1, imm2):
    """numpy semantics for CoreSim: in0=Dprev [P,N], in1=c [P,N], s0=BIG."""
    Pp, Nn = in0.shape
    big = np.broadcast_to(np.asarray(s0, np.float32), (Pp,)).astype(np.float32)
    dprev_sh = np.concatenate([big[:, None], in0[:, :-1]], axis=1)
    m = np.minimum(dprev_sh, in0)
    out = np.empty_like(in0)
    state = big.copy()
    for j in range(Nn):
        state = np.minimum(m[:, j], state) + in1[:, j]
        out[:, j] = state
    return out


def _dtw_steady() -> UopConfig:
    u = UopConfig()
    u.enable_input(InpSel.SRC_0, 1)  # chain0 = Dprev[j]
    u.enable_input(InpSel.SRC_1, 2)  # chain1 = c[j]
    u.enable_input(InpSel.CONST_0, 3)  # chain2 = BIG (seed reads)
    u.enable_input(InpSel.ZERO, 4)  # chain3 = 0.0 (seed reads)
    u.require_inp0 = ENABLE
    u.require_inp1 = ENABLE
    u.repeat_count = 0
    u.trigger = (Trigger.SRC_TENSOR_DONE, Trigger.NONE, Trigger.NONE)
    u.next_uop = (0, 0, 0)
    u.enable_output(OutSel.ALU_OUT, OutPath.WR0_LO)

    dp = u.datapath_config
    # b0: delay register via swap flop (BYPASS outputs a=old swap, captures b)
    dp[0].enable_alu(AluOp.BYPASS, AluInp.CURR_SWAP_OUT, AluInp.PREV_DELAY_0)
    dp[0].swap_enable = ENABLE
    dp[0].pass_through_delay(0, 1, 2, 3)
    # b1: m = min(dd, Dprev[j])
    dp[1].enable_alu(AluOp.MIN, AluInp.PREV_ALU_OUT, AluInp.PREV_DELAY_0)
    dp[1].pass_through_delay(1, 2, 3)
    # b2: S = S + c (feedback); capture m into chain4
    dp[2].enable_alu(AluOp.ADD, AluInp.CURR_ALU_OUT, AluInp.PREV_DELAY_1)
    dp[2].pass_through_delay(1, 2)
    dp[2].enable_delay_from_src(DelayInp.PREV_ALU_OUT, 4)
    # b3: Sx = S - c (= S[j-1]); capture S into chain5
    dp[3].enable_alu(AluOp.SUBTRACT, AluInp.PREV_ALU_OUT, AluInp.PREV_DELAY_1)
    dp[3].pass_through_delay(2, 4)
    dp[3].enable_delay_from_src(DelayInp.PREV_ALU_OUT, 5)
    # b4: t = m - S[j-1]
    dp[4].enable_alu(AluOp.SUBTRACT, AluInp.PREV_DELAY_4, AluInp.PREV_ALU_OUT)
    dp[4].pass_through_delay(2, 5)
    # b5: r = min(r, t) (feedback)
    dp[5].enable_alu(AluOp.MIN, AluInp.CURR_ALU_OUT, AluInp.PREV_ALU_OUT)
    dp[5].pass_through_delay(2, 5)
    # b6: out = r + S
    dp[6].enable_alu(AluOp.ADD, AluInp.PREV_ALU_OUT, AluInp.PREV_DELAY_5)
    # b7: passthrough to the output mux
    dp[7].pass_through_alu()
    return u


def _dtw_seed() -> UopConfig:
    u = UopConfig()
    u.enable_input(InpSel.SRC_0, 1)
    u.enable_input(InpSel.SRC_1, 2)
    u.enable_input(InpSel.CONST_0, 3)
    u.enable_input(InpSel.ZERO, 4)
    u.require_inp0 = DISABLE
    u.require_inp1 = DISABLE
    u.repeat_count = 1
    u.trigger = (Trigger.COUNT, Trigger.NONE, Trigger.NONE)
    u.next_uop = (1, 0, 0)

    dp = u.datapath_config
    # b0: swap <- C0 (BIG): BYPASS captures operand b into the swap flop
    dp[0].enable_alu(AluOp.BYPASS, AluInp.PREV_DELAY_2, AluInp.PREV_DELAY_2)
    dp[0].swap_enable = ENABLE
    dp[0].pass_through_delay(0, 1, 2, 3)
    dp[1].pass_through_alu()
    dp[1].pass_through_delay(1, 2, 3)
    # b2: S-state <- 0.0 (chain3)
    dp[2].enable_alu(AluOp.BYPASS, AluInp.PREV_DELAY_3, AluInp.PREV_DELAY_3)
    dp[2].pass_through_delay(1, 2)
    dp[3].pass_through_alu()
    dp[3].pass_through_delay(2)
    dp[4].pass_through_alu()
    dp[4].pass_through_delay(2)
    # b5: r-state <- C0 (BIG)
    dp[5].enable_alu(AluOp.BYPASS, AluInp.PREV_DELAY_2, AluInp.PREV_DELAY_2)
    dp[6].pass_through_alu()
    dp[7].pass_through_alu()
    return u


class _HandDveOp:
    """DveOp stand-in: compile() returns the hand-assembled DveOpSpec."""

    def __init__(self, name, spec, uops):
        self.name = name
        self.spec = spec
        self.subdim = False
        self.perf_en = {}
        self._uops = uops
        self._cache = {}

    def compile(self, ver):
        if ver not in self._cache:
            self._cache[ver] = DveOpSpec(
                name=self.name,
                opcode=dve_ops.get_dve_sub_opcode(self.name),
                uops=self._uops,
                rd1_en=has_src1(self.spec),
            )
        return self._cache[ver]


_REGISTERED = {}


def _register_dtw_rows_op():
    """Multi-row chained variant: in0/in1/out are [128, S, L] 3D APs; a
    1-cycle 'step' uop re-seeds the swap/S/r state at each SUB_DIM_DONE
    (row boundary). in0 = out shifted one row up, so row s consumes the
    row s-1 this same instruction produced (a full row of lag)."""
    if "DTW_ROWS_ANT" in _REGISTERED:
        return _REGISTERED["DTW_ROWS_ANT"]
    # representative spec: correct leaves {Src0, Src1, C0} + numpy reference
    S = scan(SAluOp.ADD, Src1)
    rep_body = S + scan(SAluOp.MIN, Src0 - C1, init=C0)
    spec = Spec(body=rep_body, reference=_dtw_row_reference)
    name = "DTW_ROWS_ANT"
    steady = _dtw_steady()
    steady.trigger = (Trigger.SRC_TENSOR_DONE, Trigger.SUB_DIM_DONE, Trigger.NONE)
    steady.next_uop = (0, 2, 0)
    seed = _dtw_seed()
    seed.next_uop = (1, 0, 0)
    step = _dtw_seed()  # re-seed state at each row boundary
    step.next_uop = (1, 0, 0)
    op = _HandDveOp(name, spec, [seed, steady, step])
    op.subdim = True
    dve_ops.OPS.append(op)
    dve_ops.CUSTOM_DVE_SPECS[name] = spec
    dve_ops._SUB_OPCODE_FOR_NAME[name] = (
        dve_ops._CUSTOM_DVE_ROW_BASE + len(dve_ops.OPS) - 1
    )
    _REGISTERED[name] = op
    return op


B, N, D, P, L = 32, 32, 12, 32, 1024
LOUT = 32
BIG = 1e30
NCORES = 8
BPC = B // NCORES  # batches per core
KD = BPC * D  # 48 stacked (b, d) contraction rows
F32 = mybir.dt.float32

_cached_nc = {}


def _build_kernel(reps: int = 1):
    if reps in _cached_nc:
        return _cached_nc[reps]

    nc = bacc.Bacc("TRN2", target_bir_lowering=False, debug=False, num_devices=NCORES)
    x_d = nc.dram_tensor("x", [KD, L], F32, kind="ExternalInput").ap()
    patts_d = nc.dram_tensor("patts", [N, D, P], F32, kind="ExternalInput").ap()
    out_d = nc.dram_tensor("out", [128, P * LOUT], F32, kind="ExternalOutput").ap()

    with tile.TileContext(nc) as tc:
        with (
            tc.tile_pool(name="singles", bufs=1) as singles,
            tc.tile_pool(name="psum4", bufs=1, space="PSUM") as psum4,
        ):
            # K layout (128 rows): 0-47 x, 48-63 const 1.0 (unused: stat=0),
            # 64-111 x^2, 112-127 const 1.0 (row 112 pairs with p2 in stat).
            xstack = singles.tile([128, L], F32)
            stat = singles.tile([128, 128, P], F32)  # [K, m=(b,n), i]
            ones_stage = singles.tile([D, N, P], F32)
            pst = singles.tile([D, N, P], F32)  # patts in [d, n, i]
            ones12 = singles.tile([D, 1], F32)
            p2row = singles.tile([1, N * P], F32)
            m0 = singles.tile([128, L], F32)  # row-0 m: [0, BIG, ...]
            dfull = singles.tile([128, P, L], F32)  # all DP rows

            # ---- prologue
            nc.vector.memset(xstack, 1.0)
            nc.sync.dma_start(out=xstack[0:KD, :], in_=x_d[:, :])
            nc.vector.tensor_mul(xstack[64 : 64 + KD, :], xstack[0:KD, :], xstack[0:KD, :])

            nc.vector.memset(stat, 0.0)
            nc.vector.memset(ones_stage, 1.0)
            nc.vector.memset(ones12, 1.0)
            # patts[n,d,i] -> [d, n, i] staging (also reused per-batch-block)
            patts_T = bass.AP(
                tensor=patts_d.tensor,
                offset=patts_d.offset,
                ap=[[P, D], [D * P, N], [1, P]],
            )
            nc.sync.dma_start(out=pst, in_=patts_T)
            for b in range(BPC):
                nc.sync.dma_start(
                    out=stat[D * b : D * (b + 1), N * b : N * (b + 1), :], in_=patts_T
                )
            nc.vector.tensor_scalar_mul(stat[0:KD], stat[0:KD], -2.0)
            for b in range(BPC):
                # block-diag ones rows for the x^2 contraction (DMA: engine
                # memset can't start at unaligned partitions)
                nc.sync.dma_start(
                    out=stat[64 + D * b : 64 + D * (b + 1), N * b : N * (b + 1), :],
                    in_=ones_stage[:, :, :],
                )
            # p2[n,i] = sum_d patts^2 -> stat row 112, replicated per batch block
            nc.vector.tensor_mul(pst, pst, pst)
            p2p = psum4.tile([1, N * P], F32, tag="pt")
            for h in range(2):
                nc.tensor.matmul(
                    out=p2p[:, 512 * h : 512 * (h + 1)],
                    lhsT=ones12,
                    rhs=pst[:, :, :].rearrange("p n i -> p (n i)")[:, 512 * h : 512 * (h + 1)],
                    start=True,
                    stop=True,
                )
            nc.vector.tensor_copy(p2row, p2p)
            for b in range(BPC):
                nc.sync.dma_start(
                    out=stat[112:113, N * b : N * (b + 1), :],
                    in_=p2row[:, :].rearrange("p (n i) -> p n i", n=N),
                )

            nc.vector.memset(m0, BIG)
            nc.vector.memset(m0[:, 0:1], 0.0)

            dtw_op = _register_dtw_rows_op()

            # ---- DP over rows, batched 4 rows per PSUM fill (8 banks); one
            # chained multi-row custom op per group (reps>1: slope timing only)
            GB = 4
            for _rep in range(reps):
                for g in range(P // GB):
                    ptile = psum4.tile([128, GB, L], F32, tag="pt")
                    for r in range(GB):
                        for h in range(2):
                            nc.tensor.matmul(
                                out=ptile[:, r, 512 * h : 512 * (h + 1)],
                                lhsT=stat[:, :, GB * g + r],
                                rhs=xstack[:, 512 * h : 512 * (h + 1)],
                                start=True,
                                stop=True,
                            )
                    if g == 0:
                        # row 0: m = [0, BIG, ...] constant; plain scan
                        nc.vector.tensor_tensor_scan(
                            out=dfull[:, 0, :],
                            data0=m0[:, :],
                            data1=ptile[:, 0, :],
                            initial=BIG,
                            op0=mybir.AluOpType.min,
                            op1=mybir.AluOpType.add,
                        )
                        # rows 1-3 chained in one op (row s reads row s-1)
                        nc.vector._custom_dve(
                            dtw_op,
                            out=dfull[:, 1:GB, :],
                            in0=dfull[:, 0 : GB - 1, :],
                            in1=ptile[:, 1:GB, :],
                            s0=BIG,
                            s1=0.0,
                        )
                    else:
                        i0 = GB * g
                        nc.vector._custom_dve(
                            dtw_op,
                            out=dfull[:, i0 : i0 + GB, :],
                            in0=dfull[:, i0 - 1 : i0 + GB - 1, :],
                            in1=ptile[:, :, :],
                            s0=BIG,
                            s1=0.0,
                        )

            nc.sync.dma_start(out=out_d[:, :], in_=dfull[:, :, L - LOUT : L])

    nc.compile()
    _cached_nc[reps] = nc
    return nc


def kernel(x: np.ndarray, patts: np.ndarray) -> np.ndarray:
    nc = _build_kernel()
    patts_np = np.ascontiguousarray(patts, dtype=np.float32)
    in_maps = []
    for c in range(NCORES):
        xc = np.ascontiguousarray(
            x[BPC * c : BPC * (c + 1)], dtype=np.float32
        ).reshape(KD, L)
        in_maps.append({"x": xc, "patts": patts_np})
    res = run_bass_kernel_spmd(nc, in_maps, list(range(NCORES)))
    parts = [res.results[c]["out"].reshape(BPC, N, P, LOUT) for c in range(NCORES)]
    return np.concatenate(parts, axis=0)
